# revision 3
# baseline (speedup 1.0000x reference)
"""Trainium2 Bass kernel for nn_DiagonalTraining (ragged per-anti-diagonal linear).

Math (reference): for each batch image x[b] (SxS) and each anti-diagonal
i (elements x[b, r, i-r], r=0..i), apply a per-diagonal linear layer:
  out[b,i,q] = sum_{r<=i} x[b,r,i-r] * W[i,q,r] + bias[i,q]   (q <= i)
and scatter back: y[b,q,i-q] = out[b,i,q]; positions with r+c >= S keep x.

Distribution: diagonal i -> core i%8, slot j=i//8 (64 slots per core,
balanced by construction). Host packs, per (core, slot), an augmented
matrix whose rows are the contraction axis r:
  [ D^T | V ]  with D^T[r,b]=x[b,r,i-r], V[r,q]=W[i,q,r]  (r,q < ni=i+1)
plus one extra row [ 1...1 | bias ] so the bias-add is a free extra
contraction row, zero-padded to a core-independent size NJ=8*(j+1)
(>= ni for every core) so the SPMD program is identical on all cores.
Device: per slot, stream row-chunks (<=128 rows) and accumulate
  psum[32, NJ] += chunk[:, :32].T @ chunk[:, 32:]
on the tensor engine, then DMA psum to a packed output blob. Host
scatters the blobs back into a copy of x.

Only the live (lower-triangular) part of W is shipped/read (~25 MB/core
vs 512 MB full W) — the kernel is HBM-bound on exactly those bytes.
"""

import sys

for _p in ("/opt/trn_rl_repo", "/opt/pypackages"):
    if _p not in sys.path:
        sys.path.append(_p)

import numpy as np

import concourse.bass as bass  # noqa: F401  (registers AP types)
import concourse.tile as tile
from concourse import bacc, mybir
from concourse.bass_utils import run_bass_kernel_spmd

B = 32          # batch
S = 512         # seq len / number of diagonals
N_CORES = 8
N_SLOTS = S // N_CORES  # 64 slots per core
DCOL = B        # width of the D^T block (batch on matmul M axis)

# ---- static layout (identical on every core) ----
_SLOT_NJ = [8 * (j + 1) for j in range(N_SLOTS)]
_SLOT_OFF = []          # element offset of slot region in the input blob
_OUT_OFF = []           # element offset of slot region in the output blob
_off = 0
_ooff = 0
for _j in range(N_SLOTS):
    _NJ = _SLOT_NJ[_j]
    _SLOT_OFF.append(_off)
    _OUT_OFF.append(_ooff)
    _off += (_NJ + 1) * (DCOL + _NJ)
    _ooff += B * _NJ
BLOB_ELEMS = _off        # 6,275,328  (~25.1 MB fp32)
OUT_ELEMS = _ooff        # 532,480    (~2.1 MB fp32)

_compiled_nc = None


def _build_program():
    global _compiled_nc
    if _compiled_nc is not None:
        return _compiled_nc

    from contextlib import ExitStack

    nc = bacc.Bacc("TRN2", target_bir_lowering=False, debug=False)
    f32 = mybir.dt.float32
    blob = nc.dram_tensor("blob", [BLOB_ELEMS], f32, kind="ExternalInput").ap()
    outb = nc.dram_tensor("outblob", [OUT_ELEMS], f32, kind="ExternalOutput").ap()

    with tile.TileContext(nc) as tc, ExitStack() as ctx:
        full_pool = ctx.enter_context(tc.tile_pool(name="full", bufs=3))
        rem_pool = ctx.enter_context(tc.tile_pool(name="rem", bufs=3))
        out_pool = ctx.enter_context(tc.tile_pool(name="out", bufs=3))
        psum_pool = ctx.enter_context(
            tc.tile_pool(name="psum", bufs=4, space="PSUM")
        )

        for j in range(N_SLOTS):
            NJ = _SLOT_NJ[j]
            wd = DCOL + NJ
            rows = NJ + 1
            nfull = rows // 128
            rem = rows % 128  # never 0 (rows is odd)
            base = _SLOT_OFF[j]

            psum_t = psum_pool.tile([B, NJ], f32)
            n_mm = nfull + 1
            mm = 0
            if nfull:
                t3 = full_pool.tile([128, nfull, wd], f32)
                src = blob[base : base + 128 * nfull * wd].rearrange(
                    "(c p w) -> p c w", c=nfull, p=128, w=wd
                )
                nc.sync.dma_start(t3[:], src)
                for c in range(nfull):
                    nc.tensor.matmul(
                        psum_t[:],
                        t3[:, c, 0:DCOL],
                        t3[:, c, DCOL:],
                        start=(mm == 0),
                        stop=(mm == n_mm - 1),
                    )
                    mm += 1
            t2 = rem_pool.tile([rem, wd], f32)
            src2 = blob[base + 128 * nfull * wd : base + rows * wd].rearrange(
                "(p w) -> p w", p=rem, w=wd
            )
            nc.sync.dma_start(t2[:], src2)
            nc.tensor.matmul(
                psum_t[:],
                t2[:, 0:DCOL],
                t2[:, DCOL:],
                start=(mm == 0),
                stop=True,
            )
            out_t = out_pool.tile([B, NJ], f32)
            nc.vector.tensor_copy(out_t[:], psum_t[:])
            dst = outb[_OUT_OFF[j] : _OUT_OFF[j] + B * NJ].rearrange(
                "(p w) -> p w", p=B, w=NJ
            )
            nc.gpsimd.dma_start(dst, out_t[:])

    nc.compile()
    _compiled_nc = nc
    return nc


def _pack_core(k, x, W, bias):
    blob = np.zeros(BLOB_ELEMS, np.float32)
    for j in range(N_SLOTS):
        i = N_CORES * j + k
        ni = i + 1
        NJ = _SLOT_NJ[j]
        wd = DCOL + NJ
        reg = blob[_SLOT_OFF[j] : _SLOT_OFF[j] + (NJ + 1) * wd].reshape(NJ + 1, wd)
        r = np.arange(ni)
        reg[:ni, :DCOL] = x[:, r, i - r].T            # D^T[r, b]
        reg[NJ, :DCOL] = 1.0                          # ones row -> bias add
        reg[:ni, DCOL : DCOL + ni] = W[i, :ni, :ni].T  # V[r, q]
        reg[NJ, DCOL : DCOL + ni] = bias[i, :ni]
    return blob


def kernel(x, W, b):
    x = np.asarray(x, np.float32)
    W = np.asarray(W, np.float32)
    b = np.asarray(b, np.float32)

    nc = _build_program()
    in_maps = [{"blob": _pack_core(k, x, W, b)} for k in range(N_CORES)]
    res = run_bass_kernel_spmd(nc, in_maps, list(range(N_CORES)))

    y = x.copy()
    for k in range(N_CORES):
        ob = res.results[k]["outblob"]
        for j in range(N_SLOTS):
            i = N_CORES * j + k
            ni = i + 1
            NJ = _SLOT_NJ[j]
            o = ob[_OUT_OFF[j] : _OUT_OFF[j] + B * NJ].reshape(B, NJ)
            q = np.arange(ni)
            y[:, q, i - q] = o[:, :ni]
    return y
